# revision 4
# baseline (speedup 1.0000x reference)
"""Trainium2 Bass kernel for MiniMaxText01 Lightning Attention.

Full inputs in, full output out. Sharding: heads across 8 cores
(8 heads/core x 2 batches = 16 (b,h) streams per core).

Per-core on-device algorithm (per (b,h), per 256-block i, all matmuls on PE):
  qk^T   [n,m]   = k_blk @ q_blk^T                (bf16, 2 MMs)
  qkm    [n,m]   = qk^T * decay_mask              (DVE, psum->sbuf bf16)
  o^T    [e,m]   = kv_prev^T-reduce + v^T @ qkm   (3 MMs into one PSUM)
  kv_new [d,e]   = bd*kv_prev (diag fp32 MM) + kd_blk^T @ v_blk (2 bf16 MMs)
Host pre-computes transposed/decay-scaled operands (qT, q_decay*qT, kT,
k_decay*k, v) in bf16 plus fp32 decay-mask tables; host transposes the
[e,s] outputs back to [s,e] and upcasts to fp32.
"""

import math
import sys

sys.path.insert(0, "/opt/trn_rl_repo")

import numpy as np
import ml_dtypes

BF16 = ml_dtypes.bfloat16

B, H, S, D = 2, 64, 4096, 64
BLOCK = 256
NB = S // BLOCK            # 16 blocks per sequence
NCORES = 8
HPC = H // NCORES          # 8 heads per core
NPAIR = 8                  # (b, head-pair) streams per core: 2 batches x 4 pairs
LAYER_IDX = 5
NUM_HIDDEN_LAYERS = 32
HALF = S // 2


def get_slopes(head_dim):
    equ = lambda x: 1 / 2 ** (8 / x)
    log2 = math.log2(head_dim)
    if log2.is_integer():
        return [equ(head_dim) ** i for i in range(1, head_dim + 1)]
    lower = 2 ** math.floor(log2)
    upper = 2 ** math.ceil(log2)
    ls = get_slopes(lower)
    us = get_slopes(upper)
    return ls + us[::2][: head_dim - lower]


def _slopes() -> np.ndarray:
    s = np.asarray(get_slopes(D), dtype=np.float32)
    s = s * (1.0 - LAYER_IDX / (NUM_HIDDEN_LAYERS - 1) + 1e-05)
    return s.astype(np.float32)  # [H]


def _decay_tables():
    """Per-head decay tables matching reference numerics exactly (fp32)."""
    sl = _slopes().astype(np.float64)  # [H]
    arr = np.arange(1, BLOCK + 1, dtype=np.float64)  # 1..256
    qdec = np.exp(-sl[:, None] * arr[None, :])              # [H, 256] pos m
    kdec = np.exp(-sl[:, None] * (BLOCK - arr)[None, :])    # [H, 256] pos n
    bd = np.exp(-sl * BLOCK)                                # [H]
    idx = arr[:, None] - arr[None, :]                       # m - n
    # decayT[h, n, m] = exp(-s*(m-n)) for m>=n else 0
    decT = np.where(
        idx.T[None] >= 0, np.exp(-sl[:, None, None] * np.maximum(idx.T[None], 0.0)), 0.0
    )  # [H, 256(n), 256(m)]
    return (
        qdec.astype(np.float32),
        kdec.astype(np.float32),
        bd.astype(np.float32),
        decT.astype(np.float32),
    )


def prep_core_inputs(c, q, k, v):
    """Build the per-core input map (numpy arrays) for core c."""
    qdec, kdec, bd, decT = _DECAY
    h0 = c * HPC
    sl_q = qdec[h0 : h0 + HPC]   # [8, 256]
    sl_k = kdec[h0 : h0 + HPC]
    qc = q[:, h0 : h0 + HPC]     # [2, 8, S, D] f32
    kc = k[:, h0 : h0 + HPC]
    vc = v[:, h0 : h0 + HPC]

    nrep = S // BLOCK
    qdec_s = np.tile(sl_q, (1, nrep))  # [8, S]
    kdec_s = np.tile(sl_k, (1, nrep))

    def pairT(x):  # [2,8,S,D] -> [8 pairs, 128(hh*64+d), S]
        x = x.reshape(B, 4, 2, S, D).transpose(0, 1, 2, 4, 3)  # [b,j,hh,D,S]
        return np.ascontiguousarray(x.reshape(NPAIR, 2 * D, S))

    def pairTile(x):  # [2,8,S,D] -> [8, 2(hh), 128(p), 32*64] with s=(t,p)
        x = x.reshape(B, 4, 2, NB * 2, 128, D).transpose(0, 1, 2, 4, 3, 5)
        return np.ascontiguousarray(x.reshape(NPAIR, 2, 128, HALF))

    qT = pairT(qc).astype(BF16)
    qdT = pairT(qc * qdec_s[None, :, :, None].reshape(1, HPC, S, 1)).astype(BF16)
    kT = pairT(kc).astype(BF16)
    kd = pairTile(kc * kdec_s[None, :, :, None].reshape(1, HPC, S, 1)).astype(BF16)
    vt = pairTile(vc).astype(BF16)

    # decay-mask [8 local heads, 128, 384]:
    #   cols 0:256  = decT[n in 0:128,   m in 0:256]
    #   cols 256:384= decT[n in 128:256, m in 128:256]
    dT = decT[h0 : h0 + HPC]
    msk = np.concatenate([dT[:, 0:128, :], dT[:, 128:256, 128:256]], axis=2)
    msk = np.ascontiguousarray(msk.astype(np.float32))

    bdg = (bd[h0 : h0 + HPC, None, None] * np.eye(D, dtype=np.float32)[None]).astype(
        np.float32
    )

    return {
        "qT": qT,
        "qdT": qdT,
        "kT": kT,
        "kd": kd,
        "v": vt,
        "msk": msk,
        "bdg": bdg,
    }


_DECAY = _decay_tables()


def gather_outputs(per_core_o):
    """per_core_o: list of [8, 128, S] bf16 -> [B, H, S, D] f32."""
    out = np.empty((B, H, S, D), dtype=np.float32)
    for c, oc in enumerate(per_core_o):
        x = np.asarray(oc).astype(np.float32)  # [8, 128, S]
        x = x.reshape(B, 4, 2, D, S).transpose(0, 1, 2, 4, 3)  # [b,j,hh,S,D]
        out[:, c * HPC : (c + 1) * HPC] = x.reshape(B, HPC, S, D)
    return out


# ---------------------------------------------------------------- bass kernel


def build_nc():
    import concourse.bass as bass
    import concourse.mybir as mybir
    import concourse.tile as tile
    from concourse import bacc
    from contextlib import ExitStack

    BF = mybir.dt.bfloat16
    F32 = mybir.dt.float32
    COPY = mybir.ActivationFunctionType.Copy

    nc = bacc.Bacc(
        "TRN2", target_bir_lowering=False, debug=False, num_devices=NCORES
    )
    qT_d = nc.dram_tensor("qT", [NPAIR, 128, S], BF, kind="ExternalInput").ap()
    qdT_d = nc.dram_tensor("qdT", [NPAIR, 128, S], BF, kind="ExternalInput").ap()
    kT_d = nc.dram_tensor("kT", [NPAIR, 128, S], BF, kind="ExternalInput").ap()
    kd_d = nc.dram_tensor("kd", [NPAIR, 2, 128, HALF], BF, kind="ExternalInput").ap()
    v_d = nc.dram_tensor("v", [NPAIR, 2, 128, HALF], BF, kind="ExternalInput").ap()
    msk_d = nc.dram_tensor("msk", [HPC, 128, 384], F32, kind="ExternalInput").ap()
    bdg_d = nc.dram_tensor("bdg", [HPC, 64, 64], F32, kind="ExternalInput").ap()
    o_d = nc.dram_tensor("o", [NPAIR, 128, S], BF, kind="ExternalOutput").ap()

    with tile.TileContext(nc) as tc, ExitStack() as ctx:
        consts = ctx.enter_context(tc.tile_pool(name="consts", bufs=1))
        big = ctx.enter_context(tc.tile_pool(name="big", bufs=2))
        qkmp = ctx.enter_context(tc.tile_pool(name="qkmp", bufs=4))
        kvp = ctx.enter_context(tc.tile_pool(name="kvp", bufs=4))
        psq = ctx.enter_context(tc.tile_pool(name="psq", bufs=2, space="PSUM"))
        pso = ctx.enter_context(tc.tile_pool(name="pso", bufs=2, space="PSUM"))
        pskv = ctx.enter_context(tc.tile_pool(name="pskv", bufs=2, space="PSUM"))

        mt = []
        bdt = []
        for lh in range(HPC):
            hh = lh % 2
            P0 = 64 * hh
            m_ = consts.tile([128, 384], F32, name=f"mask{lh}", tag=f"mask{lh}")
            nc.sync.dma_start(m_[:], msk_d[lh])
            mt.append(m_)
            # bdg placed at the stream's partition base so the diag-MM
            # operands share base_partition
            b_ = consts.tile([128, 64], F32, name=f"bd{lh}", tag=f"bd{lh}")
            nc.sync.dma_start(b_[P0 : P0 + 64, :], bdg_d[lh])
            bdt.append(b_)

        for p in range(NPAIR):
            j = p % 4
            qT_sb = big.tile([128, S], BF, name="qT_sb", tag="qT_sb")
            nc.sync.dma_start(qT_sb[:], qT_d[p])
            qdT_sb = big.tile([128, S], BF, name="qdT_sb", tag="qdT_sb")
            nc.sync.dma_start(qdT_sb[:], qdT_d[p])
            kT_sb = big.tile([128, S], BF, name="kT_sb", tag="kT_sb")
            nc.sync.dma_start(kT_sb[:], kT_d[p])
            kd_sb = big.tile([128, S], BF, name="kd_sb", tag="kd_sb")
            nc.sync.dma_start(kd_sb[:, 0:HALF], kd_d[p, 0])
            nc.sync.dma_start(kd_sb[:, HALF:S], kd_d[p, 1])
            v_sb = big.tile([128, S], BF, name="v_sb", tag="v_sb")
            nc.sync.dma_start(v_sb[:, 0:HALF], v_d[p, 0])
            nc.sync.dma_start(v_sb[:, HALF:S], v_d[p, 1])
            o_sb = big.tile([128, S], BF, name="o_sb", tag="o_sb")

            kvf = [None, None]
            kvb = [None, None]
            for i in range(NB):
                for hh in range(2):
                    lh = 2 * j + hh
                    P0 = 64 * hh
                    s0 = BLOCK * i
                    t0 = HALF * hh + 128 * i  # free offset of v/kd chunk0
                    first = i == 0

                    qk_ps = psq.tile([128, 384], F32, name="qk_ps", tag="qk_ps")
                    nc.tensor.matmul(
                        qk_ps[:, 0:256],
                        kT_sb[P0 : P0 + 64, s0 : s0 + 128],
                        qT_sb[P0 : P0 + 64, s0 : s0 + 256],
                        start=True,
                        stop=True,
                    )
                    nc.tensor.matmul(
                        qk_ps[:, 256:384],
                        kT_sb[P0 : P0 + 64, s0 + 128 : s0 + 256],
                        qT_sb[P0 : P0 + 64, s0 + 128 : s0 + 256],
                        start=True,
                        stop=True,
                    )
                    qkm = qkmp.tile([128, 384], BF, name="qkm", tag="qkm")
                    nc.vector.tensor_mul(qkm[:], qk_ps[:], mt[lh][:])

                    o_ps = pso.tile([128, 256], F32, name="o_ps", tag="o_ps")
                    o_sl = o_ps[P0 : P0 + 64, :]
                    if not first:
                        nc.tensor.matmul(
                            o_sl,
                            kvb[hh][P0 : P0 + 64, :],
                            qdT_sb[P0 : P0 + 64, s0 : s0 + 256],
                            start=True,
                            stop=False,
                        )
                    nc.tensor.matmul(
                        o_sl,
                        v_sb[:, t0 : t0 + 64],
                        qkm[:, 0:256],
                        start=first,
                        stop=False,
                    )
                    nc.tensor.matmul(
                        o_ps[P0 : P0 + 64, 128:256],
                        v_sb[:, t0 + 64 : t0 + 128],
                        qkm[:, 256:384],
                        start=False,
                        stop=True,
                    )
                    nc.scalar.activation(
                        o_sb[P0 : P0 + 64, s0 : s0 + 256], o_sl, COPY
                    )

                    kv_ps = pskv.tile([128, 64], F32, name="kv_ps", tag="kv_ps")
                    kv_sl = kv_ps[P0 : P0 + 64, :]
                    if not first:
                        nc.tensor.matmul(
                            kv_sl,
                            bdt[lh][P0 : P0 + 64, :],
                            kvf[hh][P0 : P0 + 64, :],
                            start=True,
                            stop=False,
                        )
                    nc.tensor.matmul(
                        kv_sl,
                        kd_sb[:, t0 : t0 + 64],
                        v_sb[:, t0 : t0 + 64],
                        start=first,
                        stop=False,
                    )
                    nc.tensor.matmul(
                        kv_sl,
                        kd_sb[:, t0 + 64 : t0 + 128],
                        v_sb[:, t0 + 64 : t0 + 128],
                        start=False,
                        stop=True,
                    )
                    if i < NB - 1:
                        kvf_new = kvp.tile([128, 64], F32, name="kvf_new", tag="kvf")
                        nc.scalar.activation(
                            kvf_new[P0 : P0 + 64, :], kv_sl, COPY
                        )
                        kvb_new = kvp.tile([128, 64], BF, name="kvb_new", tag="kvb")
                        nc.vector.tensor_copy(
                            kvb_new[P0 : P0 + 64, :], kvf_new[P0 : P0 + 64, :]
                        )
                        kvf[hh] = kvf_new
                        kvb[hh] = kvb_new

            nc.sync.dma_start(o_d[p], o_sb[:])

    nc.compile()
    return nc


_NC = None


def _get_nc():
    global _NC
    if _NC is None:
        _NC = build_nc()
    return _NC


def run_on_device(in_maps, trace=False, **kw):
    from concourse.bass_utils import run_bass_kernel_spmd

    nc = _get_nc()
    return run_bass_kernel_spmd(nc, in_maps, list(range(NCORES)), trace=trace, **kw)


def kernel(query_states, key_states, value_states, attention_mask):
    q = np.asarray(query_states, dtype=np.float32)
    k = np.asarray(key_states, dtype=np.float32)
    v = np.asarray(value_states, dtype=np.float32)
    am = np.asarray(attention_mask)
    v = v * am[:, None, :, None].astype(np.float32)

    in_maps = [prep_core_inputs(c, q, k, v) for c in range(NCORES)]
    res = run_on_device(in_maps)
    return gather_outputs([res.results[c]["o"] for c in range(NCORES)])


# revision 11
# speedup vs baseline: 1.2681x; 1.2681x over previous
"""Trainium2 Bass kernel for MiniMaxText01 Lightning Attention.

Full inputs in, full output out. Sharding: heads across 8 cores
(8 heads/core x 2 batches = 16 (b,h) streams per core).

Per-core on-device algorithm (per (b,h), per 256-block i, all matmuls on PE):
  qk^T   [n,m]   = k_blk @ q_blk^T                (bf16, 2 MMs)
  qkm    [n,m]   = qk^T * decay_mask              (DVE, psum->sbuf bf16)
  o^T    [e,m]   = kv_prev^T-reduce + v^T @ qkm   (3 MMs into one PSUM)
  kv_new [d,e]   = bd*kv_prev (diag fp32 MM) + kd_blk^T @ v_blk (2 bf16 MMs)
Host pre-computes transposed/decay-scaled operands (qT, q_decay*qT, kT,
k_decay*k, v) in bf16 plus fp32 decay-mask tables; host transposes the
[e,s] outputs back to [s,e] and upcasts to fp32.
"""

import math
import sys

sys.path.insert(0, "/opt/trn_rl_repo")

import numpy as np
import ml_dtypes

BF16 = ml_dtypes.bfloat16

B, H, S, D = 2, 64, 4096, 64
BLOCK = 256
NB = S // BLOCK            # 16 blocks per sequence
NCORES = 8
HPC = H // NCORES          # 8 heads per core
NPAIR = 8                  # (b, head-pair) streams per core: 2 batches x 4 pairs
LAYER_IDX = 5
NUM_HIDDEN_LAYERS = 32
HALF = S // 2


def get_slopes(head_dim):
    equ = lambda x: 1 / 2 ** (8 / x)
    log2 = math.log2(head_dim)
    if log2.is_integer():
        return [equ(head_dim) ** i for i in range(1, head_dim + 1)]
    lower = 2 ** math.floor(log2)
    upper = 2 ** math.ceil(log2)
    ls = get_slopes(lower)
    us = get_slopes(upper)
    return ls + us[::2][: head_dim - lower]


def _slopes() -> np.ndarray:
    s = np.asarray(get_slopes(D), dtype=np.float32)
    s = s * (1.0 - LAYER_IDX / (NUM_HIDDEN_LAYERS - 1) + 1e-05)
    return s.astype(np.float32)  # [H]


def _decay_tables():
    """Per-head decay tables matching reference numerics exactly (fp32)."""
    sl = _slopes().astype(np.float64)  # [H]
    arr = np.arange(1, BLOCK + 1, dtype=np.float64)  # 1..256
    qdec = np.exp(-sl[:, None] * arr[None, :])              # [H, 256] pos m
    kdec = np.exp(-sl[:, None] * (BLOCK - arr)[None, :])    # [H, 256] pos n
    bd = np.exp(-sl * BLOCK)                                # [H]
    idx = arr[:, None] - arr[None, :]                       # m - n
    # decayT[h, n, m] = exp(-s*(m-n)) for m>=n else 0
    decT = np.where(
        idx.T[None] >= 0, np.exp(-sl[:, None, None] * np.maximum(idx.T[None], 0.0)), 0.0
    )  # [H, 256(n), 256(m)]
    return (
        qdec.astype(np.float32),
        kdec.astype(np.float32),
        bd.astype(np.float32),
        decT.astype(np.float32),
    )


def prep_core_inputs(c, q, k, v):
    """Build the per-core input map (numpy arrays) for core c."""
    qdec, kdec, bd, decT = _DECAY
    h0 = c * HPC
    sl_q = qdec[h0 : h0 + HPC]   # [8, 256]
    sl_k = kdec[h0 : h0 + HPC]
    qc = q[:, h0 : h0 + HPC]     # [2, 8, S, D] f32
    kc = k[:, h0 : h0 + HPC]
    vc = v[:, h0 : h0 + HPC]

    nrep = S // BLOCK
    qdec_s = np.tile(sl_q, (1, nrep))  # [8, S]
    kdec_s = np.tile(sl_k, (1, nrep))

    def pairT(x):  # [2,8,S,D] -> [8 pairs, 128(hh*64+d), S]
        x = x.reshape(B, 4, 2, S, D).transpose(0, 1, 2, 4, 3)  # [b,j,hh,D,S]
        return np.ascontiguousarray(x.reshape(NPAIR, 2 * D, S))

    def pairTile(x):  # [2,8,S,D] -> [8, 2(hh), 128(p), 32*64] with s=(t,p)
        x = x.reshape(B, 4, 2, NB * 2, 128, D).transpose(0, 1, 2, 4, 3, 5)
        return np.ascontiguousarray(x.reshape(NPAIR, 2, 128, HALF))

    qT = pairT(qc).astype(BF16)
    qdT = pairT(qc * qdec_s[None, :, :, None].reshape(1, HPC, S, 1)).astype(BF16)
    kT = pairT(kc).astype(BF16)
    kd = pairTile(kc * kdec_s[None, :, :, None].reshape(1, HPC, S, 1)).astype(BF16)
    vt = pairTile(vc).astype(BF16)

    # decay-mask [8 local heads, 128, 384]:
    #   cols 0:256  = decT[n in 0:128,   m in 0:256]
    #   cols 256:384= decT[n in 128:256, m in 128:256]
    dT = decT[h0 : h0 + HPC]
    msk = np.concatenate([dT[:, 0:128, :], dT[:, 128:256, 128:256]], axis=2)
    msk = np.ascontiguousarray(msk.astype(np.float32))

    # per-pair-index j: [128] vector with bd of local head 2j on partitions
    # 0:64 and head 2j+1 on 64:128 (per-partition scalar for the kv update)
    bdv = np.repeat(bd[h0 : h0 + HPC].reshape(4, 2), 64, axis=1).reshape(4, 128)
    bdv = np.ascontiguousarray(bdv.astype(np.float32))

    return {
        "qT": qT,
        "qdT": qdT,
        "kT": kT,
        "kd": kd,
        "v": vt,
        "msk": msk,
        "bdv": bdv,
    }


_DECAY = _decay_tables()


def gather_outputs(per_core_o):
    """per_core_o: list of [8, 128, S] bf16 -> [B, H, S, D] f32."""
    out = np.empty((B, H, S, D), dtype=np.float32)
    for c, oc in enumerate(per_core_o):
        x = np.asarray(oc).astype(np.float32)  # [8, 128, S]
        x = x.reshape(B, 4, 2, D, S).transpose(0, 1, 2, 4, 3)  # [b,j,hh,S,D]
        out[:, c * HPC : (c + 1) * HPC] = x.reshape(B, HPC, S, D)
    return out


# ---------------------------------------------------------------- bass kernel


def build_nc():
    import concourse.bass as bass
    import concourse.mybir as mybir
    import concourse.tile as tile
    from concourse import bacc
    from contextlib import ExitStack

    BF = mybir.dt.bfloat16
    F32 = mybir.dt.float32
    COPY = mybir.ActivationFunctionType.Copy

    nc = bacc.Bacc(
        "TRN2", target_bir_lowering=False, debug=False, num_devices=NCORES
    )
    qT_d = nc.dram_tensor("qT", [NPAIR, 128, S], BF, kind="ExternalInput").ap()
    qdT_d = nc.dram_tensor("qdT", [NPAIR, 128, S], BF, kind="ExternalInput").ap()
    kT_d = nc.dram_tensor("kT", [NPAIR, 128, S], BF, kind="ExternalInput").ap()
    kd_d = nc.dram_tensor("kd", [NPAIR, 2, 128, HALF], BF, kind="ExternalInput").ap()
    v_d = nc.dram_tensor("v", [NPAIR, 2, 128, HALF], BF, kind="ExternalInput").ap()
    msk_d = nc.dram_tensor("msk", [HPC, 128, 384], F32, kind="ExternalInput").ap()
    bdv_d = nc.dram_tensor("bdv", [4, 128], F32, kind="ExternalInput").ap()
    o_d = nc.dram_tensor("o", [NPAIR, 128, S], BF, kind="ExternalOutput").ap()

    with tile.TileContext(nc) as tc, ExitStack() as ctx:
        consts = ctx.enter_context(tc.tile_pool(name="consts", bufs=1))
        big = ctx.enter_context(tc.tile_pool(name="big", bufs=2))
        qkmp = ctx.enter_context(tc.tile_pool(name="qkmp", bufs=4))
        kvp = ctx.enter_context(tc.tile_pool(name="kvp", bufs=4))
        psq = ctx.enter_context(tc.tile_pool(name="psq", bufs=3, space="PSUM"))
        pso = ctx.enter_context(tc.tile_pool(name="pso", bufs=2, space="PSUM"))
        pskv = ctx.enter_context(tc.tile_pool(name="pskv", bufs=3, space="PSUM"))

        mt = []
        bdt = []
        for lh in range(HPC):
            m_ = consts.tile([128, 384], F32, name=f"mask{lh}", tag=f"mask{lh}")
            nc.sync.dma_start(m_[:], msk_d[lh])
            mt.append(m_)
        for j in range(4):
            b_ = consts.tile([128, 1], F32, name=f"bdv{j}", tag=f"bdv{j}")
            nc.sync.dma_start(b_[:, 0:1], bdv_d[j])
            bdt.append(b_)

        for p in range(NPAIR):
            j = p % 4
            qT_sb = big.tile([128, S], BF, name="qT_sb", tag="qT_sb")
            nc.sync.dma_start(qT_sb[:], qT_d[p])
            qdT_sb = big.tile([128, S], BF, name="qdT_sb", tag="qdT_sb")
            nc.sync.dma_start(qdT_sb[:], qdT_d[p])
            kT_sb = big.tile([128, S], BF, name="kT_sb", tag="kT_sb")
            nc.sync.dma_start(kT_sb[:], kT_d[p])
            kd_sb = big.tile([128, S], BF, name="kd_sb", tag="kd_sb")
            nc.sync.dma_start(kd_sb[:, 0:HALF], kd_d[p, 0])
            nc.sync.dma_start(kd_sb[:, HALF:S], kd_d[p, 1])
            v_sb = big.tile([128, S], BF, name="v_sb", tag="v_sb")
            nc.sync.dma_start(v_sb[:, 0:HALF], v_d[p, 0])
            nc.sync.dma_start(v_sb[:, HALF:S], v_d[p, 1])
            o_sb = big.tile([128, S], BF, name="o_sb", tag="o_sb")

            kvf = None
            kvb = None
            for i in range(NB):
                s0 = BLOCK * i
                first = i == 0

                o_ps = pso.tile([128, 256], F32, name="o_ps", tag="o_ps")
                kv_ps = pskv.tile([128, 64], F32, name="kv_ps", tag="kv_ps")

                for hh in range(2):
                    lh = 2 * j + hh
                    P0 = 64 * hh
                    t0 = HALF * hh + 128 * i  # free offset of v/kd chunk0

                    qk_ps = psq.tile([128, 384], F32, name="qk_ps", tag="qk_ps")
                    nc.tensor.matmul(
                        qk_ps[:, 0:256],
                        kT_sb[P0 : P0 + 64, s0 : s0 + 128],
                        qT_sb[P0 : P0 + 64, s0 : s0 + 256],
                        start=True,
                        stop=True,
                    )
                    nc.tensor.matmul(
                        qk_ps[:, 256:384],
                        kT_sb[P0 : P0 + 64, s0 + 128 : s0 + 256],
                        qT_sb[P0 : P0 + 64, s0 + 128 : s0 + 256],
                        start=True,
                        stop=True,
                    )
                    qkm = qkmp.tile([128, 384], BF, name="qkm", tag="qkm")
                    nc.vector.tensor_mul(qkm[:], qk_ps[:], mt[lh][:])

                    o_sl = o_ps[P0 : P0 + 64, :]
                    if not first:
                        nc.tensor.matmul(
                            o_sl,
                            kvb[P0 : P0 + 64, :],
                            qdT_sb[P0 : P0 + 64, s0 : s0 + 256],
                            start=True,
                            stop=False,
                        )
                    nc.tensor.matmul(
                        o_sl,
                        v_sb[:, t0 : t0 + 64],
                        qkm[:, 0:256],
                        start=first,
                        stop=False,
                    )
                    nc.tensor.matmul(
                        o_ps[P0 : P0 + 64, 128:256],
                        v_sb[:, t0 + 64 : t0 + 128],
                        qkm[:, 256:384],
                        start=False,
                        stop=True,
                    )

                    kv_sl = kv_ps[P0 : P0 + 64, :]
                    nc.tensor.matmul(
                        kv_sl,
                        kd_sb[:, t0 : t0 + 64],
                        v_sb[:, t0 : t0 + 64],
                        start=True,
                        stop=False,
                    )
                    nc.tensor.matmul(
                        kv_sl,
                        kd_sb[:, t0 + 64 : t0 + 128],
                        v_sb[:, t0 + 64 : t0 + 128],
                        start=False,
                        stop=True,
                    )

                # one copy for both head streams
                nc.scalar.activation(o_sb[:, s0 : s0 + 256], o_ps[:], COPY)

                if i < NB - 1:
                    kvf_new = kvp.tile([128, 64], F32, name="kvf_new", tag="kvf")
                    if first:
                        nc.scalar.activation(kvf_new[:], kv_ps[:], COPY)
                    else:
                        # kv_new = bd * kv_prev + delta   (fp32, one DVE op)
                        nc.vector.scalar_tensor_tensor(
                            kvf_new[:],
                            kvf[:],
                            bdt[j][:, 0:1],
                            kv_ps[:],
                            mybir.AluOpType.mult,
                            mybir.AluOpType.add,
                        )
                    kvb_new = kvp.tile([128, 64], BF, name="kvb_new", tag="kvb")
                    nc.gpsimd.tensor_copy(kvb_new[:], kvf_new[:])
                    kvf = kvf_new
                    kvb = kvb_new

            nc.sync.dma_start(o_d[p], o_sb[:])

    nc.compile()
    return nc


_NC = None


def _get_nc():
    global _NC
    if _NC is None:
        _NC = build_nc()
    return _NC


def run_on_device(in_maps, trace=False, **kw):
    from concourse.bass_utils import run_bass_kernel_spmd

    nc = _get_nc()
    return run_bass_kernel_spmd(nc, in_maps, list(range(NCORES)), trace=trace, **kw)


def kernel(query_states, key_states, value_states, attention_mask):
    q = np.asarray(query_states, dtype=np.float32)
    k = np.asarray(key_states, dtype=np.float32)
    v = np.asarray(value_states, dtype=np.float32)
    am = np.asarray(attention_mask)
    v = v * am[:, None, :, None].astype(np.float32)

    in_maps = [prep_core_inputs(c, q, k, v) for c in range(NCORES)]
    res = run_on_device(in_maps)
    return gather_outputs([res.results[c]["o"] for c in range(NCORES)])


# revision 16
# speedup vs baseline: 1.3409x; 1.0574x over previous
"""Trainium2 Bass kernel for MiniMaxText01 Lightning Attention.

Full inputs in, full output out. Sharding: heads across 8 cores
(8 heads/core x 2 batches = 16 (b,h) streams per core).

Per-core on-device algorithm (per (b,h), per 256-block i, all matmuls on PE):
  qk^T   [n,m]   = k_blk @ q_blk^T                (bf16, 2 MMs)
  qkm    [n,m]   = qk^T * decay_mask              (DVE, psum->sbuf bf16)
  o^T    [e,m]   = kv_prev^T-reduce + v^T @ qkm   (3 MMs into one PSUM)
  kv_new [d,e]   = bd*kv_prev (diag fp32 MM) + kd_blk^T @ v_blk (2 bf16 MMs)
Host pre-computes transposed/decay-scaled operands (qT, q_decay*qT, kT,
k_decay*k, v) in bf16 plus fp32 decay-mask tables; host transposes the
[e,s] outputs back to [s,e] and upcasts to fp32.
"""

import math
import sys

sys.path.insert(0, "/opt/trn_rl_repo")

import numpy as np
import ml_dtypes

BF16 = ml_dtypes.bfloat16

B, H, S, D = 2, 64, 4096, 64
BLOCK = 256
NB = S // BLOCK            # 16 blocks per sequence
NCORES = 8
HPC = H // NCORES          # 8 heads per core
NPAIR = 8                  # (b, head-pair) streams per core: 2 batches x 4 pairs
LAYER_IDX = 5
NUM_HIDDEN_LAYERS = 32
HALF = S // 2


def get_slopes(head_dim):
    equ = lambda x: 1 / 2 ** (8 / x)
    log2 = math.log2(head_dim)
    if log2.is_integer():
        return [equ(head_dim) ** i for i in range(1, head_dim + 1)]
    lower = 2 ** math.floor(log2)
    upper = 2 ** math.ceil(log2)
    ls = get_slopes(lower)
    us = get_slopes(upper)
    return ls + us[::2][: head_dim - lower]


def _slopes() -> np.ndarray:
    s = np.asarray(get_slopes(D), dtype=np.float32)
    s = s * (1.0 - LAYER_IDX / (NUM_HIDDEN_LAYERS - 1) + 1e-05)
    return s.astype(np.float32)  # [H]


def _decay_tables():
    """Per-head decay tables matching reference numerics exactly (fp32)."""
    sl = _slopes().astype(np.float64)  # [H]
    arr = np.arange(1, BLOCK + 1, dtype=np.float64)  # 1..256
    qdec = np.exp(-sl[:, None] * arr[None, :])              # [H, 256] pos m
    kdec = np.exp(-sl[:, None] * (BLOCK - arr)[None, :])    # [H, 256] pos n
    bd = np.exp(-sl * BLOCK)                                # [H]
    idx = arr[:, None] - arr[None, :]                       # m - n
    # decayT[h, n, m] = exp(-s*(m-n)) for m>=n else 0
    decT = np.where(
        idx.T[None] >= 0, np.exp(-sl[:, None, None] * np.maximum(idx.T[None], 0.0)), 0.0
    )  # [H, 256(n), 256(m)]
    return (
        qdec.astype(np.float32),
        kdec.astype(np.float32),
        bd.astype(np.float32),
        decT.astype(np.float32),
    )


def prep_core_inputs(c, q, k, v):
    """Build the per-core input map (numpy arrays) for core c."""
    qdec, kdec, bd, decT = _DECAY
    h0 = c * HPC
    sl_q = qdec[h0 : h0 + HPC]   # [8, 256]
    sl_k = kdec[h0 : h0 + HPC]
    qc = q[:, h0 : h0 + HPC]     # [2, 8, S, D] f32
    kc = k[:, h0 : h0 + HPC]
    vc = v[:, h0 : h0 + HPC]

    nrep = S // BLOCK
    qdec_s = np.tile(sl_q, (1, nrep))  # [8, S]
    kdec_s = np.tile(sl_k, (1, nrep))

    def pairT(x):  # [2,8,S,D] -> [8 pairs, 128(hh*64+d), S]
        x = x.reshape(B, 4, 2, S, D).transpose(0, 1, 2, 4, 3)  # [b,j,hh,D,S]
        return np.ascontiguousarray(x.reshape(NPAIR, 2 * D, S))

    def pairTile(x):  # [2,8,S,D] -> [8, 2(hh), 128(p), 32*64] with s=(t,p)
        x = x.reshape(B, 4, 2, NB * 2, 128, D).transpose(0, 1, 2, 4, 3, 5)
        return np.ascontiguousarray(x.reshape(NPAIR, 2, 128, HALF))

    qT = pairT(qc).astype(BF16)
    qdT = pairT(qc * qdec_s[None, :, :, None].reshape(1, HPC, S, 1)).astype(BF16)
    kT = pairT(kc).astype(BF16)
    kd = pairTile(kc * kdec_s[None, :, :, None].reshape(1, HPC, S, 1)).astype(BF16)
    vt = pairTile(vc).astype(BF16)

    # decay-mask [8 local heads, 128, 384]:
    #   cols 0:256  = decT[n in 0:128,   m in 0:256]
    #   cols 256:384= decT[n in 128:256, m in 128:256]
    dT = decT[h0 : h0 + HPC]
    msk = np.concatenate([dT[:, 0:128, :], dT[:, 128:256, 128:256]], axis=2)
    msk = np.ascontiguousarray(msk.astype(np.float32))
    mskb = np.ascontiguousarray(msk.astype(BF16))

    # per-pair-index j: [128] vector with bd of local head 2j on partitions
    # 0:64 and head 2j+1 on 64:128 (per-partition scalar for the kv update)
    bdv = np.repeat(bd[h0 : h0 + HPC].reshape(4, 2), 64, axis=1).reshape(4, 128)
    bdv = np.ascontiguousarray(bdv.astype(np.float32))

    return {
        "qT": qT,
        "qdT": qdT,
        "kT": kT,
        "kd": kd,
        "v": vt,
        "msk": msk,
        "mskb": mskb,
        "bdv": bdv,
    }


_DECAY = _decay_tables()


def gather_outputs(per_core_o):
    """per_core_o: list of [8, 128, S] bf16 -> [B, H, S, D] f32."""
    out = np.empty((B, H, S, D), dtype=np.float32)
    for c, oc in enumerate(per_core_o):
        x = np.asarray(oc).astype(np.float32)  # [8, 128, S]
        x = x.reshape(B, 4, 2, D, S).transpose(0, 1, 2, 4, 3)  # [b,j,hh,S,D]
        out[:, c * HPC : (c + 1) * HPC] = x.reshape(B, HPC, S, D)
    return out


# ---------------------------------------------------------------- bass kernel


def build_nc():
    import concourse.bass as bass
    import concourse.mybir as mybir
    import concourse.tile as tile
    from concourse import bacc
    from contextlib import ExitStack

    BF = mybir.dt.bfloat16
    F32 = mybir.dt.float32
    COPY = mybir.ActivationFunctionType.Copy

    nc = bacc.Bacc(
        "TRN2", target_bir_lowering=False, debug=False, num_devices=NCORES
    )
    qT_d = nc.dram_tensor("qT", [NPAIR, 128, S], BF, kind="ExternalInput").ap()
    qdT_d = nc.dram_tensor("qdT", [NPAIR, 128, S], BF, kind="ExternalInput").ap()
    kT_d = nc.dram_tensor("kT", [NPAIR, 128, S], BF, kind="ExternalInput").ap()
    kd_d = nc.dram_tensor("kd", [NPAIR, 2, 128, HALF], BF, kind="ExternalInput").ap()
    v_d = nc.dram_tensor("v", [NPAIR, 2, 128, HALF], BF, kind="ExternalInput").ap()
    msk_d = nc.dram_tensor("msk", [HPC, 128, 384], F32, kind="ExternalInput").ap()
    mskb_d = nc.dram_tensor("mskb", [HPC, 128, 384], BF, kind="ExternalInput").ap()
    bdv_d = nc.dram_tensor("bdv", [4, 128], F32, kind="ExternalInput").ap()
    o_d = nc.dram_tensor("o", [NPAIR, 128, S], BF, kind="ExternalOutput").ap()

    with tile.TileContext(nc) as tc, ExitStack() as ctx:
        consts = ctx.enter_context(tc.tile_pool(name="consts", bufs=1))
        big = ctx.enter_context(tc.tile_pool(name="big", bufs=2))
        qkmp = ctx.enter_context(tc.tile_pool(name="qkmp", bufs=4))
        kvp = ctx.enter_context(tc.tile_pool(name="kvp", bufs=4))
        psq = ctx.enter_context(tc.tile_pool(name="psq", bufs=3, space="PSUM"))
        pso = ctx.enter_context(tc.tile_pool(name="pso", bufs=2, space="PSUM"))
        pskv = ctx.enter_context(tc.tile_pool(name="pskv", bufs=3, space="PSUM"))

        mt = []
        bdt = []
        for lh in range(HPC):
            if lh % 2 == 0:  # even stream: DVE mask path (fp32 mask)
                m_ = consts.tile([128, 384], F32, name=f"mask{lh}", tag=f"mask{lh}")
                nc.sync.dma_start(m_[:], msk_d[lh])
            else:  # odd stream: ACT-copy + GpSimd-mult path (bf16 mask)
                m_ = consts.tile([128, 384], BF, name=f"maskb{lh}", tag=f"maskb{lh}")
                nc.sync.dma_start(m_[:], mskb_d[lh])
            mt.append(m_)
        for j in range(4):
            b_ = consts.tile([128, 1], F32, name=f"bdv{j}", tag=f"bdv{j}")
            nc.sync.dma_start(b_[:, 0:1], bdv_d[j])
            bdt.append(b_)

        for p in range(NPAIR):
            j = p % 4
            qT_sb = big.tile([128, S], BF, name="qT_sb", tag="qT_sb")
            nc.sync.dma_start(qT_sb[:], qT_d[p])
            qdT_sb = big.tile([128, S], BF, name="qdT_sb", tag="qdT_sb")
            nc.sync.dma_start(qdT_sb[:], qdT_d[p])
            kT_sb = big.tile([128, S], BF, name="kT_sb", tag="kT_sb")
            nc.sync.dma_start(kT_sb[:], kT_d[p])
            kd_sb = big.tile([128, S], BF, name="kd_sb", tag="kd_sb")
            nc.sync.dma_start(kd_sb[:, 0:HALF], kd_d[p, 0])
            nc.sync.dma_start(kd_sb[:, HALF:S], kd_d[p, 1])
            v_sb = big.tile([128, S], BF, name="v_sb", tag="v_sb")
            nc.sync.dma_start(v_sb[:, 0:HALF], v_d[p, 0])
            nc.sync.dma_start(v_sb[:, HALF:S], v_d[p, 1])
            o_sb = big.tile([128, S], BF, name="o_sb", tag="o_sb")

            kvf = None
            kvb = None
            for i in range(NB):
                s0 = BLOCK * i
                first = i == 0
                t0s = [HALF * hh + 128 * i for hh in range(2)]
                P0s = [0, 64]

                o_ps = pso.tile([128, 256], F32, name="o_ps", tag="o_ps")
                kv_ps = pskv.tile([128, 64], F32, name="kv_ps", tag="kv_ps")

                # qk matmuls, alternating row halves across the two streams
                qk_pss = []
                for hh in range(2):
                    P0 = P0s[hh]
                    qk_ps = psq.tile([128, 384], F32, name="qk_ps", tag="qk_ps")
                    qk_pss.append(qk_ps)
                    nc.tensor.matmul(
                        qk_ps[:, 0:256],
                        kT_sb[P0 : P0 + 64, s0 : s0 + 128],
                        qT_sb[P0 : P0 + 64, s0 : s0 + 256],
                        start=True,
                        stop=True,
                    )
                for hh in range(2):
                    P0 = P0s[hh]
                    nc.tensor.matmul(
                        qk_pss[hh][:, 256:384],
                        kT_sb[P0 : P0 + 64, s0 + 128 : s0 + 256],
                        qT_sb[P0 : P0 + 64, s0 + 128 : s0 + 256],
                        start=True,
                        stop=True,
                    )

                # decay-mask multiply: stream 0 on DVE, stream 1 on ACT+GpSimd
                qkms = []
                for hh in range(2):
                    lh = 2 * j + hh
                    qkm = qkmp.tile([128, 384], BF, name="qkm", tag=f"qkm{hh}")
                    qkms.append(qkm)
                    if hh == 0:
                        nc.vector.tensor_mul(qkm[:], qk_pss[hh][:], mt[lh][:])
                    else:
                        qkc = qkmp.tile([128, 384], BF, name="qkc", tag="qkc")
                        nc.scalar.activation(qkc[:], qk_pss[hh][:], COPY)
                        nc.gpsimd.tensor_mul(qkm[:], qkc[:], mt[lh][:])

                # kv delta matmuls, alternating column halves
                for c in range(2):
                    for hh in range(2):
                        P0, t0 = P0s[hh], t0s[hh]
                        nc.tensor.matmul(
                            kv_ps[P0 : P0 + 64, :],
                            kd_sb[:, t0 + 64 * c : t0 + 64 * c + 64],
                            v_sb[:, t0 + 64 * c : t0 + 64 * c + 64],
                            start=(c == 0),
                            stop=(c == 1),
                        )

                # inter (kv state) matmuls
                if not first:
                    for hh in range(2):
                        P0 = P0s[hh]
                        nc.tensor.matmul(
                            o_ps[P0 : P0 + 64, :],
                            kvb[P0 : P0 + 64, :],
                            qdT_sb[P0 : P0 + 64, s0 : s0 + 256],
                            start=True,
                            stop=False,
                        )

                # intra matmuls, alternating column halves
                for hh in range(2):
                    P0, t0 = P0s[hh], t0s[hh]
                    nc.tensor.matmul(
                        o_ps[P0 : P0 + 64, :],
                        v_sb[:, t0 : t0 + 64],
                        qkms[hh][:, 0:256],
                        start=first,
                        stop=False,
                    )
                for hh in range(2):
                    P0, t0 = P0s[hh], t0s[hh]
                    nc.tensor.matmul(
                        o_ps[P0 : P0 + 64, 128:256],
                        v_sb[:, t0 + 64 : t0 + 128],
                        qkms[hh][:, 256:384],
                        start=False,
                        stop=True,
                    )

                # one copy for both head streams
                nc.scalar.activation(o_sb[:, s0 : s0 + 256], o_ps[:], COPY)

                if i < NB - 1:
                    kvf_new = kvp.tile([128, 64], F32, name="kvf_new", tag="kvf")
                    if first:
                        nc.scalar.activation(kvf_new[:], kv_ps[:], COPY)
                    else:
                        # kv_new = bd * kv_prev + delta   (fp32, one DVE op)
                        nc.vector.scalar_tensor_tensor(
                            kvf_new[:],
                            kvf[:],
                            bdt[j][:, 0:1],
                            kv_ps[:],
                            mybir.AluOpType.mult,
                            mybir.AluOpType.add,
                        )
                    kvb_new = kvp.tile([128, 64], BF, name="kvb_new", tag="kvb")
                    nc.gpsimd.tensor_copy(kvb_new[:], kvf_new[:])
                    kvf = kvf_new
                    kvb = kvb_new

            nc.sync.dma_start(o_d[p], o_sb[:])

    nc.compile()
    return nc


_NC = None


def _get_nc():
    global _NC
    if _NC is None:
        _NC = build_nc()
    return _NC


def run_on_device(in_maps, trace=False, **kw):
    from concourse.bass_utils import run_bass_kernel_spmd

    nc = _get_nc()
    return run_bass_kernel_spmd(nc, in_maps, list(range(NCORES)), trace=trace, **kw)


def kernel(query_states, key_states, value_states, attention_mask):
    q = np.asarray(query_states, dtype=np.float32)
    k = np.asarray(key_states, dtype=np.float32)
    v = np.asarray(value_states, dtype=np.float32)
    am = np.asarray(attention_mask)
    v = v * am[:, None, :, None].astype(np.float32)

    in_maps = [prep_core_inputs(c, q, k, v) for c in range(NCORES)]
    res = run_on_device(in_maps)
    return gather_outputs([res.results[c]["o"] for c in range(NCORES)])


# revision 18
# speedup vs baseline: 1.6707x; 1.2460x over previous
"""Trainium2 Bass kernel for MiniMaxText01 Lightning Attention.

Full inputs in, full output out. Sharding: heads across 8 cores
(8 heads/core x 2 batches = 16 (b,h) streams per core).

Per-core on-device algorithm (per (b,h), per 256-block i, all matmuls on PE):
  qk^T   [n,m]   = k_blk @ q_blk^T                (bf16, 2 MMs)
  qkm    [n,m]   = qk^T * decay_mask              (DVE, psum->sbuf bf16)
  o^T    [e,m]   = kv_prev^T-reduce + v^T @ qkm   (3 MMs into one PSUM)
  kv_new [d,e]   = bd*kv_prev (diag fp32 MM) + kd_blk^T @ v_blk (2 bf16 MMs)
Host pre-computes transposed/decay-scaled operands (qT, q_decay*qT, kT,
k_decay*k, v) in bf16 plus fp32 decay-mask tables; host transposes the
[e,s] outputs back to [s,e] and upcasts to fp32.
"""

import math
import sys

sys.path.insert(0, "/opt/trn_rl_repo")

import numpy as np
import ml_dtypes

BF16 = ml_dtypes.bfloat16

B, H, S, D = 2, 64, 4096, 64
BLOCK = 256
NB = S // BLOCK            # 16 blocks per sequence
NCORES = 8
HPC = H // NCORES          # 8 heads per core
NPAIR = 8                  # (b, head-pair) streams per core: 2 batches x 4 pairs
LAYER_IDX = 5
NUM_HIDDEN_LAYERS = 32
HALF = S // 2


def get_slopes(head_dim):
    equ = lambda x: 1 / 2 ** (8 / x)
    log2 = math.log2(head_dim)
    if log2.is_integer():
        return [equ(head_dim) ** i for i in range(1, head_dim + 1)]
    lower = 2 ** math.floor(log2)
    upper = 2 ** math.ceil(log2)
    ls = get_slopes(lower)
    us = get_slopes(upper)
    return ls + us[::2][: head_dim - lower]


def _slopes() -> np.ndarray:
    s = np.asarray(get_slopes(D), dtype=np.float32)
    s = s * (1.0 - LAYER_IDX / (NUM_HIDDEN_LAYERS - 1) + 1e-05)
    return s.astype(np.float32)  # [H]


def _decay_tables():
    """Per-head decay tables matching reference numerics exactly (fp32)."""
    sl = _slopes().astype(np.float64)  # [H]
    arr = np.arange(1, BLOCK + 1, dtype=np.float64)  # 1..256
    qdec = np.exp(-sl[:, None] * arr[None, :])              # [H, 256] pos m
    kdec = np.exp(-sl[:, None] * (BLOCK - arr)[None, :])    # [H, 256] pos n
    bd = np.exp(-sl * BLOCK)                                # [H]
    idx = arr[:, None] - arr[None, :]                       # m - n
    # decayT[h, n, m] = exp(-s*(m-n)) for m>=n else 0
    decT = np.where(
        idx.T[None] >= 0, np.exp(-sl[:, None, None] * np.maximum(idx.T[None], 0.0)), 0.0
    )  # [H, 256(n), 256(m)]
    return (
        qdec.astype(np.float32),
        kdec.astype(np.float32),
        bd.astype(np.float32),
        decT.astype(np.float32),
    )


def prep_core_inputs(c, q, k, v):
    """Build the per-core input map (numpy arrays) for core c."""
    qdec, kdec, bd, decT = _DECAY
    h0 = c * HPC
    sl_q = qdec[h0 : h0 + HPC]   # [8, 256]
    sl_k = kdec[h0 : h0 + HPC]
    qc = q[:, h0 : h0 + HPC]     # [2, 8, S, D] f32
    kc = k[:, h0 : h0 + HPC]
    vc = v[:, h0 : h0 + HPC]

    nrep = S // BLOCK
    qdec_s = np.tile(sl_q, (1, nrep))  # [8, S]
    kdec_s = np.tile(sl_k, (1, nrep))

    def pairT(x):  # [2,8,S,D] -> [8 pairs, 128(hh*64+d), S]
        x = x.reshape(B, 4, 2, S, D).transpose(0, 1, 2, 4, 3)  # [b,j,hh,D,S]
        return np.ascontiguousarray(x.reshape(NPAIR, 2 * D, S))

    def pairTile(x):  # [2,8,S,D] -> [8, 2(hh), 128(p), 32*64] with s=(t,p)
        x = x.reshape(B, 4, 2, NB * 2, 128, D).transpose(0, 1, 2, 4, 3, 5)
        return np.ascontiguousarray(x.reshape(NPAIR, 2, 128, HALF))

    qT = pairT(qc).astype(BF16)
    qdT = pairT(qc * qdec_s[None, :, :, None].reshape(1, HPC, S, 1)).astype(BF16)
    kT = pairT(kc).astype(BF16)
    kd = pairTile(kc * kdec_s[None, :, :, None].reshape(1, HPC, S, 1)).astype(BF16)
    vt = pairTile(vc).astype(BF16)

    # decay-mask [8 local heads, 128, 384]:
    #   cols 0:256  = decT[n in 0:128,   m in 0:256]
    #   cols 256:384= decT[n in 128:256, m in 128:256]
    dT = decT[h0 : h0 + HPC]
    msk = np.concatenate([dT[:, 0:128, :], dT[:, 128:256, 128:256]], axis=2)
    msk = np.ascontiguousarray(msk.astype(np.float32))
    mskb = np.ascontiguousarray(msk.astype(BF16))

    # per-pair-index j: [128] vector with bd of local head 2j on partitions
    # 0:64 and head 2j+1 on 64:128 (per-partition scalar for the kv update)
    bdv = np.repeat(bd[h0 : h0 + HPC].reshape(4, 2), 64, axis=1).reshape(4, 128)
    bdv = np.ascontiguousarray(bdv.astype(np.float32))

    return {
        "qT": qT,
        "qdT": qdT,
        "kT": kT,
        "kd": kd,
        "v": vt,
        "msk": msk,
        "mskb": mskb,
        "bdv": bdv,
    }


_DECAY = _decay_tables()


def gather_outputs(per_core_o):
    """per_core_o: list of [8, 128, S] bf16 -> [B, H, S, D] f32."""
    out = np.empty((B, H, S, D), dtype=np.float32)
    for c, oc in enumerate(per_core_o):
        x = np.asarray(oc).astype(np.float32)  # [8, 128, S]
        x = x.reshape(B, 4, 2, D, S).transpose(0, 1, 2, 4, 3)  # [b,j,hh,S,D]
        out[:, c * HPC : (c + 1) * HPC] = x.reshape(B, HPC, S, D)
    return out


# ---------------------------------------------------------------- bass kernel


def build_nc():
    import concourse.bass as bass
    import concourse.mybir as mybir
    import concourse.tile as tile
    from concourse import bacc
    from contextlib import ExitStack

    BF = mybir.dt.bfloat16
    F32 = mybir.dt.float32
    COPY = mybir.ActivationFunctionType.Copy

    nc = bacc.Bacc(
        "TRN2", target_bir_lowering=False, debug=False, num_devices=NCORES
    )
    qT_d = nc.dram_tensor("qT", [NPAIR, 128, S], BF, kind="ExternalInput").ap()
    qdT_d = nc.dram_tensor("qdT", [NPAIR, 128, S], BF, kind="ExternalInput").ap()
    kT_d = nc.dram_tensor("kT", [NPAIR, 128, S], BF, kind="ExternalInput").ap()
    kd_d = nc.dram_tensor("kd", [NPAIR, 2, 128, HALF], BF, kind="ExternalInput").ap()
    v_d = nc.dram_tensor("v", [NPAIR, 2, 128, HALF], BF, kind="ExternalInput").ap()
    msk_d = nc.dram_tensor("msk", [HPC, 128, 384], F32, kind="ExternalInput").ap()
    mskb_d = nc.dram_tensor("mskb", [HPC, 128, 384], BF, kind="ExternalInput").ap()
    bdv_d = nc.dram_tensor("bdv", [4, 128], F32, kind="ExternalInput").ap()
    o_d = nc.dram_tensor("o", [NPAIR, 128, S], BF, kind="ExternalOutput").ap()

    with tile.TileContext(nc) as tc, ExitStack() as ctx:
        consts = ctx.enter_context(tc.tile_pool(name="consts", bufs=1))
        big = ctx.enter_context(tc.tile_pool(name="big", bufs=2))
        qkmp = ctx.enter_context(tc.tile_pool(name="qkmp", bufs=4))
        kvp = ctx.enter_context(tc.tile_pool(name="kvp", bufs=4))
        psq = ctx.enter_context(tc.tile_pool(name="psq", bufs=4, space="PSUM"))
        pso = ctx.enter_context(tc.tile_pool(name="pso", bufs=2, space="PSUM"))
        pskv = ctx.enter_context(tc.tile_pool(name="pskv", bufs=2, space="PSUM"))

        mt = []
        bdt = []
        for lh in range(HPC):
            if lh % 2 == 0:  # even stream: DVE mask path (fp32 mask)
                m_ = consts.tile([128, 384], F32, name=f"mask{lh}", tag=f"mask{lh}")
                nc.sync.dma_start(m_[:], msk_d[lh])
            else:  # odd stream: ACT-copy + GpSimd-mult path (bf16 mask)
                m_ = consts.tile([128, 384], BF, name=f"maskb{lh}", tag=f"maskb{lh}")
                nc.sync.dma_start(m_[:], mskb_d[lh])
            mt.append(m_)
        for j in range(4):
            b_ = consts.tile([128, 1], F32, name=f"bdv{j}", tag=f"bdv{j}")
            nc.sync.dma_start(b_[:, 0:1], bdv_d[j])
            bdt.append(b_)

        for p in range(NPAIR):
            j = p % 4
            qT_sb = big.tile([128, S], BF, name="qT_sb", tag="qT_sb")
            nc.sync.dma_start(qT_sb[:], qT_d[p])
            qdT_sb = big.tile([128, S], BF, name="qdT_sb", tag="qdT_sb")
            nc.sync.dma_start(qdT_sb[:], qdT_d[p])
            kT_sb = big.tile([128, S], BF, name="kT_sb", tag="kT_sb")
            nc.sync.dma_start(kT_sb[:], kT_d[p])
            kd_sb = big.tile([128, S], BF, name="kd_sb", tag="kd_sb")
            nc.sync.dma_start(kd_sb[:, 0:HALF], kd_d[p, 0])
            nc.sync.dma_start(kd_sb[:, HALF:S], kd_d[p, 1])
            v_sb = big.tile([128, S], BF, name="v_sb", tag="v_sb")
            nc.sync.dma_start(v_sb[:, 0:HALF], v_d[p, 0])
            nc.sync.dma_start(v_sb[:, HALF:S], v_d[p, 1])
            o_sb = big.tile([128, S], BF, name="o_sb", tag="o_sb")

            kvf = None
            kvb = None
            for i in range(NB):
                s0 = BLOCK * i
                first = i == 0
                t0s = [HALF * hh + 128 * i for hh in range(2)]
                P0s = [0, 64]

                o_ps = pso.tile([128, 256], F32, name="o_ps", tag="o_ps")
                kv_ps = pskv.tile([128, 64], F32, name="kv_ps", tag="kv_ps")

                # qk matmuls, alternating row halves across the two streams
                qk_pss = []
                for hh in range(2):
                    P0 = P0s[hh]
                    qk_ps = psq.tile([128, 384], F32, name="qk_ps", tag="qk_ps")
                    qk_pss.append(qk_ps)
                    nc.tensor.matmul(
                        qk_ps[:, 0:256],
                        kT_sb[P0 : P0 + 64, s0 : s0 + 128],
                        qT_sb[P0 : P0 + 64, s0 : s0 + 256],
                        start=True,
                        stop=True,
                    )
                for hh in range(2):
                    P0 = P0s[hh]
                    nc.tensor.matmul(
                        qk_pss[hh][:, 256:384],
                        kT_sb[P0 : P0 + 64, s0 + 128 : s0 + 256],
                        qT_sb[P0 : P0 + 64, s0 + 128 : s0 + 256],
                        start=True,
                        stop=True,
                    )

                # decay-mask multiply: stream 0 on DVE, stream 1 on ACT+GpSimd
                qkms = []
                for hh in range(2):
                    lh = 2 * j + hh
                    qkm = qkmp.tile([128, 384], BF, name="qkm", tag=f"qkm{hh}")
                    qkms.append(qkm)
                    if hh == 0:
                        nc.vector.tensor_mul(qkm[:], qk_pss[hh][:], mt[lh][:])
                    else:
                        qkc = qkmp.tile([128, 384], BF, name="qkc", tag="qkc")
                        nc.scalar.activation(qkc[:], qk_pss[hh][:], COPY)
                        nc.vector.tensor_mul(qkm[:], qkc[:], mt[lh][:])

                # kv delta matmuls, alternating column halves
                for c in range(2):
                    for hh in range(2):
                        P0, t0 = P0s[hh], t0s[hh]
                        nc.tensor.matmul(
                            kv_ps[P0 : P0 + 64, :],
                            kd_sb[:, t0 + 64 * c : t0 + 64 * c + 64],
                            v_sb[:, t0 + 64 * c : t0 + 64 * c + 64],
                            start=(c == 0),
                            stop=(c == 1),
                        )

                # inter (kv state) matmuls
                if not first:
                    for hh in range(2):
                        P0 = P0s[hh]
                        nc.tensor.matmul(
                            o_ps[P0 : P0 + 64, :],
                            kvb[P0 : P0 + 64, :],
                            qdT_sb[P0 : P0 + 64, s0 : s0 + 256],
                            start=True,
                            stop=False,
                        )

                # intra matmuls, alternating column halves
                for hh in range(2):
                    P0, t0 = P0s[hh], t0s[hh]
                    nc.tensor.matmul(
                        o_ps[P0 : P0 + 64, :],
                        v_sb[:, t0 : t0 + 64],
                        qkms[hh][:, 0:256],
                        start=first,
                        stop=False,
                    )
                for hh in range(2):
                    P0, t0 = P0s[hh], t0s[hh]
                    nc.tensor.matmul(
                        o_ps[P0 : P0 + 64, 128:256],
                        v_sb[:, t0 + 64 : t0 + 128],
                        qkms[hh][:, 256:384],
                        start=False,
                        stop=True,
                    )

                # one copy for both head streams
                nc.scalar.activation(o_sb[:, s0 : s0 + 256], o_ps[:], COPY)

                if i < NB - 1:
                    kvf_new = kvp.tile([128, 64], F32, name="kvf_new", tag="kvf")
                    if first:
                        nc.scalar.activation(kvf_new[:], kv_ps[:], COPY)
                    else:
                        # kv_new = bd * kv_prev + delta   (fp32, one DVE op)
                        nc.vector.scalar_tensor_tensor(
                            kvf_new[:],
                            kvf[:],
                            bdt[j][:, 0:1],
                            kv_ps[:],
                            mybir.AluOpType.mult,
                            mybir.AluOpType.add,
                        )
                    kvb_new = kvp.tile([128, 64], BF, name="kvb_new", tag="kvb")
                    nc.gpsimd.tensor_copy(kvb_new[:], kvf_new[:])
                    kvf = kvf_new
                    kvb = kvb_new

            nc.sync.dma_start(o_d[p], o_sb[:])

    nc.compile()
    return nc


_NC = None


def _get_nc():
    global _NC
    if _NC is None:
        _NC = build_nc()
    return _NC


def run_on_device(in_maps, trace=False, **kw):
    from concourse.bass_utils import run_bass_kernel_spmd

    nc = _get_nc()
    return run_bass_kernel_spmd(nc, in_maps, list(range(NCORES)), trace=trace, **kw)


def kernel(query_states, key_states, value_states, attention_mask):
    q = np.asarray(query_states, dtype=np.float32)
    k = np.asarray(key_states, dtype=np.float32)
    v = np.asarray(value_states, dtype=np.float32)
    am = np.asarray(attention_mask)
    v = v * am[:, None, :, None].astype(np.float32)

    in_maps = [prep_core_inputs(c, q, k, v) for c in range(NCORES)]
    res = run_on_device(in_maps)
    return gather_outputs([res.results[c]["o"] for c in range(NCORES)])
